# revision 20
# baseline (speedup 1.0000x reference)
"""Trainium2 Bass kernel for a 16-head causal MHA block (B=4, S=2048, D=1024).

Sharding: tensor-parallel over heads — 8 cores x 2 heads each. The
reference's final reshape is a raw [B,H,S,hd]->[B,S,H*hd] view (no head
transpose), so each output row s' draws from exactly one head
(h = s'//128): the output projection is head-local and no collective is
needed. Core c computes output rows [128*2c, 128*(2c+2)) of every batch.

Device-side per core:
  qhT/khT[h]  = (x[b] @ W[h]).T        via lhsT=W-tiles, rhs=xT-tiles (f32r)
  scoresT     = khT.T @ qhT            [k_pos, q_pos], 2 heads row-packed
  ET          = exp(scoresT/8)         ACT, bf16 out, causal mask on diagonal
  attn_outT   = vh_aug.T @ ET          bf16 MM; vh_aug has a ones column
  normalize   = recip_approx(rowsum) -> gpsimd partition_broadcast -> DVE mul
                (no PE involvement; the old DVE reciprocal stalled the PE)
  out rows    = sum_t Apair_t @ Wo2    K=128 via a shifted attn replica

q/k/v stream in as bf16 (halves HBM traffic); the pre-softmax path
(qh/kh, scores) stays f32/f32r, the post-softmax path is bf16.
"""

import numpy as np

B, S, D, H, HD = 4, 2048, 1024, 16, 64
NCORES = 8
HPC = H // NCORES        # heads per core = 2
SH = S // H              # output rows per head = 128
NDT = D // 128           # 8 contraction tiles for projections
NSC = S // 512           # 4 s-chunks of 512
NKT = S // 128           # 16 k-tiles
DRIVE = 5                # pipeline steps advanced per proj chunk emitted

_CACHE = {}


def _build_nc():
    import concourse.mybir as mybir
    import concourse.tile as tile
    from concourse import bacc

    F32 = mybir.dt.float32
    F32R = mybir.dt.float32r
    BF16 = mybir.dt.bfloat16
    EXP = mybir.ActivationFunctionType.Exp

    nc = bacc.Bacc("TRN2", target_bir_lowering=False, debug=False,
                   num_devices=NCORES)

    qT = nc.dram_tensor("qT", [B, D, S], BF16, kind="ExternalInput").ap()
    kT = nc.dram_tensor("kT", [B, D, S], BF16, kind="ExternalInput").ap()
    vT = nc.dram_tensor("vT", [B, D, S], BF16, kind="ExternalInput").ap()
    wq = nc.dram_tensor("wq", [128, D], BF16, kind="ExternalInput").ap()
    wk = nc.dram_tensor("wk", [128, D], BF16, kind="ExternalInput").ap()
    wv = nc.dram_tensor("wv", [128, D], BF16, kind="ExternalInput").ap()
    # wo2[64g+e, t*1024+d] = Wo[64*(2t+g)+e, d]
    wo2 = nc.dram_tensor("wo2", [128, (H // 2) * D], BF16,
                         kind="ExternalInput").ap()
    masks = nc.dram_tensor("masks", [128, 4 * 512], BF16,
                           kind="ExternalInput").ap()
    ident = nc.dram_tensor("ident", [128, 128], BF16, kind="ExternalInput").ap()
    out = nc.dram_tensor("out", [B, HPC, SH, D], F32, kind="ExternalOutput").ap()

    with tile.TileContext(nc) as tc:
        with tc.tile_pool(name="const", bufs=1) as cst, \
             tc.tile_pool(name="stage", bufs=1) as stage, \
             tc.tile_pool(name="xT", bufs=26) as pxT, \
             tc.tile_pool(name="qhT", bufs=2) as pqh, \
             tc.tile_pool(name="khT", bufs=2) as pkh, \
             tc.tile_pool(name="vv", bufs=3) as pvv, \
             tc.tile_pool(name="vha", bufs=64) as pvha, \
             tc.tile_pool(name="et", bufs=3) as pet, \
             tc.tile_pool(name="attn", bufs=6) as patt, \
             tc.tile_pool(name="rv", bufs=3) as prv, \
             tc.tile_pool(name="rvb", bufs=3) as prvb, \
             tc.tile_pool(name="osb", bufs=3) as posb, \
             tc.tile_pool(name="psp", bufs=2, space="PSUM") as psp, \
             tc.tile_pool(name="pss", bufs=2, space="PSUM") as pss, \
             tc.tile_pool(name="pso", bufs=2, space="PSUM") as pso:

            # ---- constants ------------------------------------------------
            w_sb = {}
            for name, src in (("q", wq), ("k", wk), ("v", wv)):
                raw = stage.tile([128, D], BF16, tag="wstage")
                nc.sync.dma_start(raw[:], src[:])
                cooked = cst.tile([128, D], BF16, tag="w" + name)
                nc.vector.tensor_copy(cooked[:], raw[:])
                w_sb[name] = cooked
            ident_raw = stage.tile([128, 128], BF16, tag="identr")
            nc.sync.dma_start(ident_raw[:], ident[:])
            ident_sb = cst.tile([128, 128], BF16, tag="ident")
            nc.vector.tensor_copy(ident_sb[:], ident_raw[:])
            masks_sb = cst.tile([128, 4 * 512], BF16, tag="masks")
            wo_sb = cst.tile([128, (H // 2) * D], BF16, tag="wo2")
            late_consts = [False]

            def emit_late_consts():
                # wo2 (2MB) is first needed by batch0 out-proj; deferring its
                # DMA keeps the ring clear for the startup projections
                if not late_consts[0]:
                    late_consts[0] = True
                    nc.sync.dma_start(masks_sb[:], masks[:])
                    nc.sync.dma_start(wo_sb[:], wo2[:])

            st_xt = {}

            def emit_proj_chunk(b, tname, sc, st):
                """One (tensor, 512-wide s-chunk) projection for batch b."""
                src = {"q": qT, "k": kT, "v": vT}[tname]
                w = w_sb[tname]
                sp = sc // 2
                if (tname, sp) not in st_xt or st_xt[(tname, sp)][0] != b:
                    xts = []
                    for dt in range(NDT):
                        xt = pxT.tile([128, 1024], BF16, tag="xT",
                                      name=f"xt_{b}_{tname}_{sp}_{dt}")
                        nc.sync.dma_start(
                            xt[:], src[b, dt * 128:(dt + 1) * 128,
                                       sp * 1024:(sp + 1) * 1024])
                        xts.append(xt)
                    st_xt[(tname, sp)] = (b, xts)
                xts = st_xt[(tname, sp)][1]
                off = (sc % 2) * 512
                ps = psp.tile([128, 512], F32, tag="psp",
                              name=f"ps_{b}_{tname}_{sc}")
                for dt in range(NDT):
                    nc.tensor.matmul(
                        ps[:], w[:, dt * 128:(dt + 1) * 128],
                        xts[dt][:, off:off + 512],
                        start=(dt == 0), stop=(dt == NDT - 1))
                if tname == "q":
                    nc.vector.tensor_copy(
                        st["qh"][:, sc * 512:(sc + 1) * 512], ps[:])
                elif tname == "k":
                    nc.vector.tensor_copy(
                        st["kh"][:, sc * 512:(sc + 1) * 512], ps[:])
                else:
                    vv = pvv.tile([128, 512], BF16, tag="vv",
                                  name=f"vv_{b}_{sc}")
                    nc.vector.tensor_copy(vv[:], ps[:])
                    for j in range(4):
                        pt = psp.tile([128, 128], BF16, tag="psp",
                                      name=f"pt_{b}_{sc}_{j}")
                        nc.tensor.transpose(
                            pt[:], vv[:, j * 128:(j + 1) * 128], ident_sb[:])
                        stl = sc * 4 + j
                        for h in range(HPC):
                            va = pvha.tile([128, 128], BF16, tag="vha",
                                           name=f"va_{b}_{stl}_{h}")
                            nc.vector.tensor_copy(
                                va[:, 0:64], pt[:, 64 * h:64 * h + 64])
                            nc.gpsimd.memset(va[:, 64:128], 1.0)
                            st["vha"][h][stl] = va

            def gen_attention_qc(b, qc, st):
                """Generator emitting one q-chunk of batch b's attention."""
                qh_t, kh_t, vh_aug, at_t = st["qh"], st["kh"], st["vha"], st["at"]
                nkt = 4 * qc + 4
                po = [pso.tile([128, 512], F32, tag="pso",
                               name=f"po_{b}_{qc}_{hh}")
                      for hh in range(HPC)]
                def emit_av(kt, ett):
                    j = kt - 4 * qc          # >=0 on diagonal tiles
                    lo = 128 * j if j > 0 else 0
                    for h in range(HPC):
                        nc.tensor.matmul(
                            po[h][:, lo:512], vh_aug[h][kt][:],
                            ett[:, h * 512 + lo:(h + 1) * 512],
                            start=(kt == 0), stop=(kt == nkt - 1))

                for kt in range(nkt):
                    j = kt - 4 * qc          # >=0 on diagonal tiles
                    pscore = pss.tile([128, 1024], F32, tag="pss",
                                      name=f"psc_{b}_{qc}_{kt}")
                    for h in range(HPC):
                        nc.tensor.matmul(
                            pscore[:, h * 512:(h + 1) * 512],
                            kh_t[64 * h:64 * h + 64,
                                 kt * 128:(kt + 1) * 128],
                            qh_t[64 * h:64 * h + 64,
                                 qc * 512:(qc + 1) * 512],
                            start=True, stop=True)
                    ett = pet.tile([128, 1024], BF16, tag="et",
                                   name=f"et_{b}_{qc}_{kt}")
                    nc.scalar.activation(ett[:], pscore[:], EXP, scale=0.125)
                    if j >= 0:
                        # triangle mask on the first valid 128 columns
                        lo = 128 * j
                        for h in range(HPC):
                            nc.gpsimd.tensor_mul(
                                ett[:, h * 512 + lo:h * 512 + lo + 128],
                                ett[:, h * 512 + lo:h * 512 + lo + 128],
                                masks_sb[:, 0:128])
                    emit_av(kt, ett)
                    yield
                # ---- normalize: no tensor-engine instructions -------------
                # po rows 64:128 hold the rowsum replicated across 64
                # partitions (ones block in vha), so the reciprocal runs
                # 64-lane-parallel and needs no partition broadcast.
                for h in range(HPC):
                    rs = prv.tile([64, 512], F32, tag="rs",
                                  name=f"rs_{b}_{qc}_{h}")
                    nc.vector.tensor_copy(rs[:], po[h][64:128, :])
                    rvb = prvb.tile([64, 512], F32, tag="rvb",
                                    name=f"rvb_{b}_{qc}_{h}")
                    with nc.allow_low_precision(reason="softmax denom"):
                        nc.vector.reciprocal_approx_fast(rvb[:], rs[:])
                    nc.vector.tensor_mul(
                        at_t[h][0:64, qc * 512:(qc + 1) * 512],
                        po[h][0:64, :],
                        rvb[:])
                # incremental shifted replica: at[64+e, s] = at[e, s+1]
                for h in range(HPC):
                    lo = qc * 512
                    nc.sync.dma_start(
                        at_t[h][64:128, max(0, lo - 1):(qc + 1) * 512 - 1],
                        at_t[h][0:64, max(1, lo):(qc + 1) * 512])
                yield

            def gen_outproj(b, st):
                for h in range(HPC):
                    a3 = st["at"][h][:].rearrange("p (u m) -> p m u", m=H)
                    for ch in range(2):
                        pop = psp.tile([128, 512], F32, tag="psp",
                                       name=f"pop_{b}_{h}_{ch}")
                        for t in range(H // 2):
                            nc.tensor.matmul(
                                pop[:], a3[:, 2 * t, :],
                                wo_sb[:, t * D + ch * 512:t * D + (ch + 1) * 512],
                                start=(t == 0), stop=(t == H // 2 - 1))
                            if t == 3:
                                yield
                        osb = posb.tile([128, 512], F32, tag="osb",
                                        name=f"osb_{b}_{h}_{ch}")
                        nc.vector.tensor_copy(osb[:], pop[:])
                        nc.sync.dma_start(
                            out[b, h, :, ch * 512:(ch + 1) * 512], osb[:])
                        yield

            # ---- software-pipelined emission: proj(b) ⊗ attn(b-1) --------
            pending = []
            op_defer = []

            def drive(n):
                while n > 0 and pending:
                    try:
                        next(pending[0])
                        n -= 1
                    except StopIteration:
                        pending.pop(0)

            for b in range(B):
                st = {
                    "qh": pqh.tile([128, S], F32R, tag="qhT", name=f"qh_{b}"),
                    "kh": pkh.tile([128, S], F32R, tag="khT", name=f"kh_{b}"),
                    "vha": [[None] * NKT for _ in range(HPC)],
                    "at": [patt.tile([128, S], BF16, tag="attn",
                                     name=f"att_{b}_{hh}")
                           for hh in range(HPC)],
                }
                for sc in range(NSC):
                    for tname in ("q", "k", "v"):
                        emit_proj_chunk(b, tname, sc, st)
                        drive(DRIVE)
                    emit_late_consts()
                    # attention q-chunk sc only needs proj chunks 0..sc
                    pending.append(gen_attention_qc(b, sc, st))
                if b in (1, 2):
                    op_defer.append(gen_outproj(b, st))
                else:
                    pending.append(gen_outproj(b, st))
            while pending or op_defer:
                if op_defer:
                    try:
                        next(op_defer[0])
                    except StopIteration:
                        op_defer.pop(0)
                drive(1)
    nc.compile()
    return nc


def _host_inputs(q, k, v, Wq, Wk, Wv, Wo):
    """Build the 8 per-core input maps."""
    import ml_dtypes
    bf = ml_dtypes.bfloat16
    f = np.float32
    qT = np.ascontiguousarray(q.transpose(0, 2, 1)).astype(bf)
    kT = np.ascontiguousarray(k.transpose(0, 2, 1)).astype(bf)
    vT = np.ascontiguousarray(v.transpose(0, 2, 1)).astype(bf)

    def pack_w(Wh2):                      # [D, 128] -> [128, D] tile-packed
        return np.ascontiguousarray(
            Wh2.reshape(NDT, 128, 128).transpose(1, 0, 2).reshape(128, D)
        ).astype(bf)

    # wo2[64g+e, t*1024+d] = Wo[64*(2t+g)+e, d]
    wo2 = np.ascontiguousarray(
        Wo.reshape(H // 2, 2, 64, D)        # t, g, e, d
          .transpose(1, 2, 0, 3)            # g, e, t, d
          .reshape(128, (H // 2) * D)
    ).astype(bf)

    m = np.zeros((128, 4 * 512), f)
    for j in range(4):
        m[:, j * 512:(j + 1) * 512] = (
            np.arange(128)[:, None] + 128 * j
            <= np.arange(512)[None, :]).astype(f)
    m = m.astype(bf)
    ident = np.eye(128, dtype=f).astype(bf)

    in_maps = []
    for c in range(NCORES):
        W2q = np.concatenate([Wq[2 * c], Wq[2 * c + 1]], axis=1)  # [D, 128]
        W2k = np.concatenate([Wk[2 * c], Wk[2 * c + 1]], axis=1)
        W2v = np.concatenate([Wv[2 * c], Wv[2 * c + 1]], axis=1)
        in_maps.append({
            "qT": qT, "kT": kT, "vT": vT,
            "wq": pack_w(W2q), "wk": pack_w(W2k), "wv": pack_w(W2v),
            "wo2": wo2, "masks": m, "ident": ident,
        })
    return in_maps


def kernel(q, k, v, Wq, Wk, Wv, Wo, _trace=False):
    from concourse.bass_utils import run_bass_kernel_spmd

    if "nc" not in _CACHE:
        _CACHE["nc"] = _build_nc()
    nc = _CACHE["nc"]

    q = np.asarray(q, np.float32)
    k = np.asarray(k, np.float32)
    v = np.asarray(v, np.float32)
    in_maps = _host_inputs(q, k, v, np.asarray(Wq, np.float32),
                           np.asarray(Wk, np.float32),
                           np.asarray(Wv, np.float32),
                           np.asarray(Wo, np.float32))
    res = run_bass_kernel_spmd(nc, in_maps, core_ids=list(range(NCORES)),
                               trace=_trace)
    OUT = np.empty((B, S, D), np.float32)
    for c in range(NCORES):
        oc = res.results[c]["out"]            # [B, HPC, SH, D]
        for b in range(B):
            for l in range(HPC):
                h = 2 * c + l
                OUT[b, h * SH:(h + 1) * SH, :] = oc[b, l]
    if _trace:
        return OUT, res
    return OUT


# revision 21
# speedup vs baseline: 1.1956x; 1.1956x over previous
"""Trainium2 Bass kernel for a 16-head causal MHA block (B=4, S=2048, D=1024).

Sharding: tensor-parallel over heads — 8 cores x 2 heads each. The
reference's final reshape is a raw [B,H,S,hd]->[B,S,H*hd] view (no head
transpose), so each output row s' draws from exactly one head
(h = s'//128): the output projection is head-local and no collective is
needed. Core c computes output rows [128*2c, 128*(2c+2)) of every batch.

Device-side per core:
  qhT/khT[h]  = (x[b] @ W[h]).T        via lhsT=W-tiles, rhs=xT-tiles (f32r)
  scoresT     = khT.T @ qhT            [k_pos, q_pos], 2 heads row-packed
  ET          = exp(scoresT/8)         ACT, bf16 out, causal mask on diagonal
  attn_outT   = vh_aug.T @ ET          bf16 MM; vh_aug has a ones column
  normalize   = recip_approx(rowsum) -> gpsimd partition_broadcast -> DVE mul
                (no PE involvement; the old DVE reciprocal stalled the PE)
  out rows    = sum_t Apair_t @ Wo2    K=128 via a shifted attn replica

q/k/v stream in as bf16 (halves HBM traffic); the pre-softmax path
(qh/kh, scores) stays f32/f32r, the post-softmax path is bf16.
"""

import numpy as np

B, S, D, H, HD = 4, 2048, 1024, 16, 64
NCORES = 8
HPC = H // NCORES        # heads per core = 2
SH = S // H              # output rows per head = 128
NDT = D // 128           # 8 contraction tiles for projections
NSC = S // 512           # 4 s-chunks of 512
NKT = S // 128           # 16 k-tiles
DRIVE = 5                # pipeline steps advanced per proj chunk emitted

_CACHE = {}


def _build_nc():
    import concourse.mybir as mybir
    import concourse.tile as tile
    from concourse import bacc

    F32 = mybir.dt.float32
    F32R = mybir.dt.float32r
    BF16 = mybir.dt.bfloat16
    EXP = mybir.ActivationFunctionType.Exp

    nc = bacc.Bacc("TRN2", target_bir_lowering=False, debug=False,
                   num_devices=NCORES)

    qT = nc.dram_tensor("qT", [B, D, S], BF16, kind="ExternalInput").ap()
    kT = nc.dram_tensor("kT", [B, D, S], BF16, kind="ExternalInput").ap()
    vT = nc.dram_tensor("vT", [B, D, S], BF16, kind="ExternalInput").ap()
    wq = nc.dram_tensor("wq", [128, D], BF16, kind="ExternalInput").ap()
    wk = nc.dram_tensor("wk", [128, D], BF16, kind="ExternalInput").ap()
    wv = nc.dram_tensor("wv", [128, D], BF16, kind="ExternalInput").ap()
    # wo2[64g+e, t*1024+d] = Wo[64*(2t+g)+e, d]
    wo2 = nc.dram_tensor("wo2", [128, (H // 2) * D], BF16,
                         kind="ExternalInput").ap()
    masks = nc.dram_tensor("masks", [128, 4 * 512], BF16,
                           kind="ExternalInput").ap()
    ident = nc.dram_tensor("ident", [128, 128], BF16, kind="ExternalInput").ap()
    out = nc.dram_tensor("out", [B, HPC, SH, D], F32, kind="ExternalOutput").ap()

    with tile.TileContext(nc) as tc:
        with tc.tile_pool(name="const", bufs=1) as cst, \
             tc.tile_pool(name="stage", bufs=1) as stage, \
             tc.tile_pool(name="xT", bufs=26) as pxT, \
             tc.tile_pool(name="qhT", bufs=2) as pqh, \
             tc.tile_pool(name="khT", bufs=2) as pkh, \
             tc.tile_pool(name="vv", bufs=3) as pvv, \
             tc.tile_pool(name="vha", bufs=64) as pvha, \
             tc.tile_pool(name="et", bufs=3) as pet, \
             tc.tile_pool(name="attn", bufs=6) as patt, \
             tc.tile_pool(name="rv", bufs=3) as prv, \
             tc.tile_pool(name="rvb", bufs=3) as prvb, \
             tc.tile_pool(name="osb", bufs=3) as posb, \
             tc.tile_pool(name="psp", bufs=2, space="PSUM") as psp, \
             tc.tile_pool(name="pss", bufs=2, space="PSUM") as pss, \
             tc.tile_pool(name="pso", bufs=2, space="PSUM") as pso:

            # ---- constants ------------------------------------------------
            w_sb = {}
            for name, src in (("q", wq), ("k", wk), ("v", wv)):
                raw = stage.tile([128, D], BF16, tag="wstage")
                nc.sync.dma_start(raw[:], src[:])
                cooked = cst.tile([128, D], BF16, tag="w" + name)
                nc.vector.tensor_copy(cooked[:], raw[:])
                w_sb[name] = cooked
            ident_raw = stage.tile([128, 128], BF16, tag="identr")
            nc.sync.dma_start(ident_raw[:], ident[:])
            ident_sb = cst.tile([128, 128], BF16, tag="ident")
            nc.vector.tensor_copy(ident_sb[:], ident_raw[:])
            masks_sb = cst.tile([128, 4 * 512], BF16, tag="masks")
            wo_sb = cst.tile([128, (H // 2) * D], BF16, tag="wo2")
            late_consts = [False]

            def emit_late_consts():
                # wo2 (2MB) is first needed by batch0 out-proj; deferring its
                # DMA keeps the ring clear for the startup projections
                if not late_consts[0]:
                    late_consts[0] = True
                    nc.sync.dma_start(masks_sb[:], masks[:])
                    nc.sync.dma_start(wo_sb[:], wo2[:])

            st_xt = {}

            def emit_proj_chunk(b, tname, sc, st):
                """One (tensor, 512-wide s-chunk) projection for batch b."""
                src = {"q": qT, "k": kT, "v": vT}[tname]
                w = w_sb[tname]
                sp = sc // 2
                if (tname, sp) not in st_xt or st_xt[(tname, sp)][0] != b:
                    xts = []
                    for dt in range(NDT):
                        xt = pxT.tile([128, 1024], BF16, tag="xT",
                                      name=f"xt_{b}_{tname}_{sp}_{dt}")
                        nc.sync.dma_start(
                            xt[:], src[b, dt * 128:(dt + 1) * 128,
                                       sp * 1024:(sp + 1) * 1024])
                        xts.append(xt)
                    st_xt[(tname, sp)] = (b, xts)
                xts = st_xt[(tname, sp)][1]
                off = (sc % 2) * 512
                ps = psp.tile([128, 512], F32, tag="psp",
                              name=f"ps_{b}_{tname}_{sc}")
                for dt in range(NDT):
                    nc.tensor.matmul(
                        ps[:], w[:, dt * 128:(dt + 1) * 128],
                        xts[dt][:, off:off + 512],
                        start=(dt == 0), stop=(dt == NDT - 1))
                if tname == "q":
                    nc.vector.tensor_copy(
                        st["qh"][:, sc * 512:(sc + 1) * 512], ps[:])
                elif tname == "k":
                    nc.vector.tensor_copy(
                        st["kh"][:, sc * 512:(sc + 1) * 512], ps[:])
                else:
                    vv = pvv.tile([128, 512], BF16, tag="vv",
                                  name=f"vv_{b}_{sc}")
                    nc.vector.tensor_copy(vv[:], ps[:])
                    for j in range(4):
                        pt = psp.tile([128, 128], BF16, tag="psp",
                                      name=f"pt_{b}_{sc}_{j}")
                        nc.tensor.transpose(
                            pt[:], vv[:, j * 128:(j + 1) * 128], ident_sb[:])
                        stl = sc * 4 + j
                        for h in range(HPC):
                            va = pvha.tile([128, 128], BF16, tag="vha",
                                           name=f"va_{b}_{stl}_{h}")
                            nc.vector.tensor_copy(
                                va[:, 0:64], pt[:, 64 * h:64 * h + 64])
                            nc.gpsimd.memset(va[:, 64:128], 1.0)
                            st["vha"][h][stl] = va

            def gen_attention_qc(b, qc, st):
                """Generator emitting one q-chunk of batch b's attention."""
                qh_t, kh_t, vh_aug, at_t = st["qh"], st["kh"], st["vha"], st["at"]
                nkt = 4 * qc + 4
                po = [pso.tile([128, 512], F32, tag="pso",
                               name=f"po_{b}_{qc}_{hh}")
                      for hh in range(HPC)]
                def emit_av(kt, ett):
                    j = kt - 4 * qc          # >=0 on diagonal tiles
                    lo = 128 * j if j > 0 else 0
                    for h in range(HPC):
                        nc.tensor.matmul(
                            po[h][:, lo:512], vh_aug[h][kt][:],
                            ett[:, h * 512 + lo:(h + 1) * 512],
                            start=(kt == 0), stop=(kt == nkt - 1))

                for kt in range(nkt):
                    j = kt - 4 * qc          # >=0 on diagonal tiles
                    pscore = pss.tile([128, 1024], F32, tag="pss",
                                      name=f"psc_{b}_{qc}_{kt}")
                    for h in range(HPC):
                        nc.tensor.matmul(
                            pscore[:, h * 512:(h + 1) * 512],
                            kh_t[64 * h:64 * h + 64,
                                 kt * 128:(kt + 1) * 128],
                            qh_t[64 * h:64 * h + 64,
                                 qc * 512:(qc + 1) * 512],
                            start=True, stop=True)
                    ett = pet.tile([128, 1024], BF16, tag="et",
                                   name=f"et_{b}_{qc}_{kt}")
                    nc.scalar.activation(ett[:], pscore[:], EXP, scale=0.125)
                    if j >= 0:
                        # triangle mask on the first valid 128 columns
                        lo = 128 * j
                        for h in range(HPC):
                            nc.vector.tensor_mul(
                                ett[:, h * 512 + lo:h * 512 + lo + 128],
                                ett[:, h * 512 + lo:h * 512 + lo + 128],
                                masks_sb[:, 0:128])
                    emit_av(kt, ett)
                    yield
                # ---- normalize: no tensor-engine instructions -------------
                # po rows 64:128 hold the rowsum replicated across 64
                # partitions (ones block in vha), so the reciprocal runs
                # 64-lane-parallel and needs no partition broadcast.
                for h in range(HPC):
                    rs = prv.tile([64, 512], F32, tag="rs",
                                  name=f"rs_{b}_{qc}_{h}")
                    nc.vector.tensor_copy(rs[:], po[h][64:128, :])
                    rvb = prvb.tile([64, 512], F32, tag="rvb",
                                    name=f"rvb_{b}_{qc}_{h}")
                    with nc.allow_low_precision(reason="softmax denom"):
                        nc.vector.reciprocal_approx_fast(rvb[:], rs[:])
                    nc.vector.tensor_mul(
                        at_t[h][0:64, qc * 512:(qc + 1) * 512],
                        po[h][0:64, :],
                        rvb[:])
                # incremental shifted replica: at[64+e, s] = at[e, s+1]
                for h in range(HPC):
                    lo = qc * 512
                    nc.sync.dma_start(
                        at_t[h][64:128, max(0, lo - 1):(qc + 1) * 512 - 1],
                        at_t[h][0:64, max(1, lo):(qc + 1) * 512])
                yield

            def gen_outproj(b, st):
                for h in range(HPC):
                    a3 = st["at"][h][:].rearrange("p (u m) -> p m u", m=H)
                    for ch in range(2):
                        pop = psp.tile([128, 512], F32, tag="psp",
                                       name=f"pop_{b}_{h}_{ch}")
                        for t in range(H // 2):
                            nc.tensor.matmul(
                                pop[:], a3[:, 2 * t, :],
                                wo_sb[:, t * D + ch * 512:t * D + (ch + 1) * 512],
                                start=(t == 0), stop=(t == H // 2 - 1))
                            if t == 3:
                                yield
                        osb = posb.tile([128, 512], F32, tag="osb",
                                        name=f"osb_{b}_{h}_{ch}")
                        nc.vector.tensor_copy(osb[:], pop[:])
                        nc.sync.dma_start(
                            out[b, h, :, ch * 512:(ch + 1) * 512], osb[:])
                        yield

            # ---- software-pipelined emission: proj(b) ⊗ attn(b-1) --------
            pending = []
            op_defer = []

            def drive(n):
                while n > 0 and pending:
                    try:
                        next(pending[0])
                        n -= 1
                    except StopIteration:
                        pending.pop(0)

            for b in range(B):
                st = {
                    "qh": pqh.tile([128, S], F32R, tag="qhT", name=f"qh_{b}"),
                    "kh": pkh.tile([128, S], F32R, tag="khT", name=f"kh_{b}"),
                    "vha": [[None] * NKT for _ in range(HPC)],
                    "at": [patt.tile([128, S], BF16, tag="attn",
                                     name=f"att_{b}_{hh}")
                           for hh in range(HPC)],
                }
                for sc in range(NSC):
                    for tname in ("q", "k", "v"):
                        emit_proj_chunk(b, tname, sc, st)
                        drive(DRIVE)
                    emit_late_consts()
                    # attention q-chunk sc only needs proj chunks 0..sc
                    pending.append(gen_attention_qc(b, sc, st))
                if b in (1, 2):
                    op_defer.append(gen_outproj(b, st))
                else:
                    pending.append(gen_outproj(b, st))
            while pending or op_defer:
                if op_defer:
                    try:
                        next(op_defer[0])
                    except StopIteration:
                        op_defer.pop(0)
                drive(1)
    nc.compile()
    return nc


def _host_inputs(q, k, v, Wq, Wk, Wv, Wo):
    """Build the 8 per-core input maps."""
    import ml_dtypes
    bf = ml_dtypes.bfloat16
    f = np.float32
    qT = np.ascontiguousarray(q.transpose(0, 2, 1)).astype(bf)
    kT = np.ascontiguousarray(k.transpose(0, 2, 1)).astype(bf)
    vT = np.ascontiguousarray(v.transpose(0, 2, 1)).astype(bf)

    def pack_w(Wh2):                      # [D, 128] -> [128, D] tile-packed
        return np.ascontiguousarray(
            Wh2.reshape(NDT, 128, 128).transpose(1, 0, 2).reshape(128, D)
        ).astype(bf)

    # wo2[64g+e, t*1024+d] = Wo[64*(2t+g)+e, d]
    wo2 = np.ascontiguousarray(
        Wo.reshape(H // 2, 2, 64, D)        # t, g, e, d
          .transpose(1, 2, 0, 3)            # g, e, t, d
          .reshape(128, (H // 2) * D)
    ).astype(bf)

    m = np.zeros((128, 4 * 512), f)
    for j in range(4):
        m[:, j * 512:(j + 1) * 512] = (
            np.arange(128)[:, None] + 128 * j
            <= np.arange(512)[None, :]).astype(f)
    m = m.astype(bf)
    ident = np.eye(128, dtype=f).astype(bf)

    in_maps = []
    for c in range(NCORES):
        W2q = np.concatenate([Wq[2 * c], Wq[2 * c + 1]], axis=1)  # [D, 128]
        W2k = np.concatenate([Wk[2 * c], Wk[2 * c + 1]], axis=1)
        W2v = np.concatenate([Wv[2 * c], Wv[2 * c + 1]], axis=1)
        in_maps.append({
            "qT": qT, "kT": kT, "vT": vT,
            "wq": pack_w(W2q), "wk": pack_w(W2k), "wv": pack_w(W2v),
            "wo2": wo2, "masks": m, "ident": ident,
        })
    return in_maps


def kernel(q, k, v, Wq, Wk, Wv, Wo, _trace=False):
    from concourse.bass_utils import run_bass_kernel_spmd

    if "nc" not in _CACHE:
        _CACHE["nc"] = _build_nc()
    nc = _CACHE["nc"]

    q = np.asarray(q, np.float32)
    k = np.asarray(k, np.float32)
    v = np.asarray(v, np.float32)
    in_maps = _host_inputs(q, k, v, np.asarray(Wq, np.float32),
                           np.asarray(Wk, np.float32),
                           np.asarray(Wv, np.float32),
                           np.asarray(Wo, np.float32))
    res = run_bass_kernel_spmd(nc, in_maps, core_ids=list(range(NCORES)),
                               trace=_trace)
    OUT = np.empty((B, S, D), np.float32)
    for c in range(NCORES):
        oc = res.results[c]["out"]            # [B, HPC, SH, D]
        for b in range(B):
            for l in range(HPC):
                h = 2 * c + l
                OUT[b, h * SH:(h + 1) * SH, :] = oc[b, l]
    if _trace:
        return OUT, res
    return OUT
